# revision 1
# baseline (speedup 1.0000x reference)
"""nn_CNUs kernel v2 — single fused NEFF, q-sharded over 8 TRN2 cores.

Each core handles QS=4 neurons (q) x all 1024 batch rows:
  prep (per q): load K rows, L2-normalize on device (Square/reduce/Sqrt/
    reciprocal/broadcast-mult), split into bf16 hi/lo, xbar-transpose to
    KnT [128=hi|lo, 4096] for the PE.
  x prep: same, producing xa=[xh;xl], xb=[xl;xh] stacks [128, 1024].
  main loop (q, bc) 32 iterations of 128 batch rows:
    - responses: 2 stacked-bf16 matmuls per 512-chunk (all 4 hi/lo cross
      terms, fp32 PSUM) -> exact-to-fp32 cosine responses
    - ACT copies each chunk PSUM->SBUF fp32 immediately (frees the PSUM
      bank so the PE never waits on the slow threshold chain)
    - top-16 threshold via DVE max8 on the SBUF copy (top-8 per chunk,
      2-level reduce); 0/1 mask via Pool is_ge (all 8 chunks, SBUF)
    - xbar transpose of the mask (f16-pair view), SP queue
    - combine: mask^T @ [M/16|1/16] in fp8 x f16 matmuls, software-
      pipelined two iterations behind (uniform softmax weights)
  host: gather q-shards, recompute rows whose selection count != 16.
DMA per core ~7MB vs ~49MB for batch-sharding; kernel A eliminated.
"""
import sys
if '/opt/trn_rl_repo' not in sys.path:
    sys.path.insert(0, '/opt/trn_rl_repo')

import numpy as np

import concourse.bacc as bacc
import concourse.mybir as mybir
import concourse.tile as tile
from concourse.bass import broadcast_tensor_aps
from concourse.bass_utils import run_bass_kernel_spmd

N_CORES = 8
BF, D, Q, MK, DELTA = 1024, 64, 32, 4096, 16
QS = Q // N_CORES          # 4 q per core
G = BF // 128              # 8 batch groups of 128 rows
KG = MK // 128             # 32 row-groups per q
NCH, CH, U1 = 8, 512, 65
MSPLIT = 768             # mask cols on DVE (2x) vs ACT (sigmoid)
SCALE = float(2 ** 30)
S_TEMP = 0.1 / 8.0

_cache = {}


def _build():
    nc = bacc.Bacc("TRN2", target_bir_lowering=False, debug=False,
                   num_devices=N_CORES)
    x_d = nc.dram_tensor("xc", [BF, D], mybir.dt.float32, kind="ExternalInput")
    k_d = nc.dram_tensor("Kc", [QS, MK, D], mybir.dt.float32, kind="ExternalInput")
    mp_d = nc.dram_tensor("Mp", [QS, 128, 32 * U1], mybir.dt.float16, kind="ExternalInput")
    w_d = nc.dram_tensor("WS", [128, QS * G * U1], mybir.dt.float32, kind="ExternalOutput")

    with tile.TileContext(nc) as tc:
        with tc.tile_pool(name="const", bufs=1) as cpool, \
             tc.tile_pool(name="kprep", bufs=2) as kpool, \
             tc.tile_pool(name="knt", bufs=1) as ktpool, \
             tc.tile_pool(name="mask", bufs=3) as maskpool, \
             tc.tile_pool(name="resp", bufs=2) as rpool, \
             tc.tile_pool(name="sel", bufs=2) as selpool, \
             tc.tile_pool(name="ps", bufs=1, space="PSUM") as psum:

            # q0's big K load first: overlaps the whole x-prep chain
            kc0 = kpool.tile([128, KG * D], mybir.dt.float32, tag="kc",
                             name="kc0")
            nc.sync.dma_start(
                out=kc0[:, :].rearrange("p (g d) -> p g d", g=KG),
                in_=k_d.ap()[0].rearrange("(g p) d -> p g d", p=128))

            # ---------------- x prep ----------------
            x32 = cpool.tile([128, G * D], mybir.dt.float32)
            nc.sync.dma_start(
                out=x32[:, :].rearrange("p (g d) -> p g d", g=G),
                in_=x_d.ap().rearrange("(g p) d -> p g d", p=128))
            xsq = cpool.tile([128, G * D], mybir.dt.float32)
            nc.scalar.activation(xsq[:, :], x32[:, :],
                                 mybir.ActivationFunctionType.Square)
            xss = cpool.tile([128, G], mybir.dt.float32)
            nc.vector.tensor_reduce(
                xss[:, :], xsq[:, :].rearrange("p (g d) -> p g d", g=G),
                axis=mybir.AxisListType.X, op=mybir.AluOpType.add,
                apply_absolute_value=False, negate=False)
            xrt = cpool.tile([128, G], mybir.dt.float32)
            nc.scalar.activation(xrt[:, :], xss[:, :],
                                 mybir.ActivationFunctionType.Sqrt)
            xrn = cpool.tile([128, G], mybir.dt.float32)
            nc.vector.reciprocal(xrn[:, :], xrt[:, :])
            xn32 = cpool.tile([128, G * D], mybir.dt.float32)
            a0, a1 = broadcast_tensor_aps(
                x32[:, :].rearrange("p (g d) -> p g d", g=G),
                xrn[:, :].rearrange("p (g u) -> p g u", u=1))
            nc.gpsimd.tensor_tensor(
                out=xn32[:, :].rearrange("p (g d) -> p g d", g=G),
                in0=a0, in1=a1, op=mybir.AluOpType.mult)
            # interleaved [hi|lo] per 128-col group: the tiled xbar
            # transpose then lands directly as the [xh; xl] stack
            sa = cpool.tile([128, G * 128], mybir.dt.bfloat16)
            sav = sa[:, :].rearrange("p (g s d) -> p g s d", s=2, d=D)
            nc.scalar.activation(
                sav[:, :, 0, :],
                xn32[:, :].rearrange("p (g d) -> p g d", g=G),
                mybir.ActivationFunctionType.Copy)
            nc.gpsimd.tensor_sub(
                sav[:, :, 1, :],
                xn32[:, :].rearrange("p (g d) -> p g d", g=G),
                sav[:, :, 0, :])
            sb = cpool.tile([128, G * 128], mybir.dt.bfloat16)
            sbv = sb[:, :].rearrange("p (g s d) -> p g s d", s=2, d=D)
            nc.scalar.activation(
                sbv[:, :, 1, :],
                xn32[:, :].rearrange("p (g d) -> p g d", g=G),
                mybir.ActivationFunctionType.Copy)
            nc.gpsimd.tensor_sub(
                sbv[:, :, 0, :],
                xn32[:, :].rearrange("p (g d) -> p g d", g=G),
                sbv[:, :, 1, :])
            xa = cpool.tile([128, BF], mybir.dt.bfloat16)   # [xh; xl]
            xb = cpool.tile([128, BF], mybir.dt.bfloat16)   # [xl; xh]
            nc.sync.dma_start_transpose(
                xa[:, :].rearrange("p (t b) -> p t b", t=G), sa[:, :])
            nc.sync.dma_start_transpose(
                xb[:, :].rearrange("p (t b) -> p t b", t=G), sb[:, :])

            stage = cpool.tile([128, QS * G * U1], mybir.dt.float32, tag="wout")

            # knt / mp resident tiles (one per q)
            knt = [ktpool.tile([128, MK], mybir.dt.bfloat16,
                               name=f"knt{q}", tag=f"knt{q}")
                   for q in range(QS)]
            mp = [ktpool.tile([128, 32 * U1], mybir.dt.float16,
                              name=f"mp{q}", tag=f"mp{q}")
                  for q in range(QS)]

            def emit_kprep(q, kc=None):
                nc.sync.dma_start(out=mp[q][:, :], in_=mp_d.ap()[q])
                if kc is None:
                    kc = kpool.tile([128, KG * D], mybir.dt.float32, tag="kc")
                    nc.sync.dma_start(
                        out=kc[:, :].rearrange("p (g d) -> p g d", g=KG),
                        in_=k_d.ap()[q].rearrange("(g p) d -> p g d", p=128))
                ksq = kpool.tile([128, KG * D], mybir.dt.float32, tag="ksq")
                nc.scalar.activation(ksq[:, :], kc[:, :],
                                     mybir.ActivationFunctionType.Square)
                kss = kpool.tile([128, KG], mybir.dt.float32, tag="kss")
                nc.vector.tensor_reduce(
                    kss[:, :], ksq[:, :].rearrange("p (g d) -> p g d", g=KG),
                    axis=mybir.AxisListType.X, op=mybir.AluOpType.add,
                    apply_absolute_value=False, negate=False)
                krt = kpool.tile([128, KG], mybir.dt.float32, tag="krt")
                nc.scalar.activation(krt[:, :], kss[:, :],
                                     mybir.ActivationFunctionType.Sqrt)
                krn = kpool.tile([128, KG], mybir.dt.float32, tag="krn")
                nc.vector.reciprocal(krn[:, :], krt[:, :])
                kn32 = kpool.tile([128, KG * D], mybir.dt.float32, tag="kn32")
                b0, b1 = broadcast_tensor_aps(
                    kc[:, :].rearrange("p (g d) -> p g d", g=KG),
                    krn[:, :].rearrange("p (g u) -> p g u", u=1))
                nc.gpsimd.tensor_tensor(
                    out=kn32[:, :].rearrange("p (g d) -> p g d", g=KG),
                    in0=b0, in1=b1, op=mybir.AluOpType.mult)
                sk = kpool.tile([128, KG * 128], mybir.dt.bfloat16, tag="sk")
                skv = sk[:, :].rearrange("p (g s d) -> p g s d", s=2, d=D)
                nc.scalar.activation(
                    skv[:, :, 0, :],
                    kn32[:, :].rearrange("p (g d) -> p g d", g=KG),
                    mybir.ActivationFunctionType.Copy)
                nc.gpsimd.tensor_sub(
                    skv[:, :, 1, :],
                    kn32[:, :].rearrange("p (g d) -> p g d", g=KG),
                    skv[:, :, 0, :])
                nc.sync.dma_start_transpose(
                    knt[q][:, :].rearrange("p (t b) -> p t b", t=KG),
                    sk[:, :])

            def emit_mm2(prev_mT, q_old, wp):
                mT8 = prev_mT[:, :].bitcast(mybir.dt.float8e4)
                k = 0
                for t in range(16):
                    for j in range(2):
                        lhsT = mT8[:, 256 * t:256 * (t + 1)].rearrange(
                            "p (b two) -> p b two", two=2)[:, :, j:j + 1]
                        rhs = mp[q_old][:, (t * 2 + j) * U1:(t * 2 + j + 1) * U1]
                        nc.tensor.matmul(wp[:, 0:U1], lhsT, rhs,
                                         start=(k == 0), stop=(k == 31))
                        k += 1

            def emit_epilogue(wp, q_old, bc_old):
                off = (q_old * G + bc_old) * U1
                nc.scalar.activation(stage[:, off:off + U1], wp[:, 0:U1],
                                     mybir.ActivationFunctionType.Copy)

            emit_kprep(0)
            # software pipeline state: iteration records
            #   rec = (rcopy, v2, bt, mask8, q, bc)   [masks pending]
            #   pend = (mT, q, bc)                    [combine pending]
            prev = None
            pend = []
            for it in range(QS * G):
                q, bc = divmod(it, G)
                if bc == 1 and q + 1 < QS:
                    emit_kprep(q + 1)

                cands = selpool.tile([128, 64], mybir.dt.float32, tag="cands")
                rcopy = rpool.tile([128, 4 * CH], mybir.dt.float32, tag="rcopy")

                # lag-1 DVE mask first: fills DVE idle before chunk 0 lands
                if prev is not None:
                    prcopy, pv2, pbt, pmask8, ppq, ppbc = prev
                    nc.vector.tensor_scalar(pmask8[:, 0:MSPLIT],
                                            prcopy[:, 0:MSPLIT],
                                            pv2[:, 7:8], None,
                                            op0=mybir.AluOpType.is_ge)

                chunks = []
                for c in range(NCH):
                    rp = psum.tile([128, CH], mybir.dt.float32, tag=f"bank{c}",
                                   name=f"bank{c}")
                    nc.tensor.matmul(rp[:, :], xa[:, bc * 128:(bc + 1) * 128],
                                     knt[q][:, CH * c:CH * (c + 1)],
                                     start=True, stop=False)
                    nc.tensor.matmul(rp[:, :], xb[:, bc * 128:(bc + 1) * 128],
                                     knt[q][:, CH * c:CH * (c + 1)],
                                     start=False, stop=True)
                    if c < 4:
                        # free banks 0-3 early for the next iteration; scan
                        # the SBUF copy to avoid PSUM port contention
                        nc.scalar.activation(rcopy[:, CH * c:CH * (c + 1)],
                                             rp[:, :],
                                             mybir.ActivationFunctionType.Copy)
                        nc.vector.max(cands[:, 8 * c:8 * (c + 1)],
                                      rcopy[:, CH * c:CH * (c + 1)])
                    else:
                        nc.vector.max(cands[:, 8 * c:8 * (c + 1)], rp[:, :])
                    chunks.append(rp)

                # lag-1 ACT mask + transpose for the previous iteration
                if prev is not None:
                    nc.scalar.activation(pmask8[:, MSPLIT:2048],
                                         prcopy[:, MSPLIT:2048],
                                         mybir.ActivationFunctionType.Sigmoid,
                                         bias=pbt[:, 0:1], scale=SCALE)
                    pm16 = pmask8[:, :].bitcast(mybir.dt.float16)
                    mT = maskpool.tile([128, 2048], mybir.dt.float16, tag="maskT")
                    nc.sync.dma_start_transpose(
                        mT[:, :].rearrange("p (t b) -> p t b", t=16),
                        pm16[:, :])
                    pend.append((mT, ppq, ppbc))

                # pipelined combine (lag 2) into bank 0 after its copy
                if len(pend) == 2:
                    pmT, pq, pbc = pend.pop(0)
                    emit_mm2(pmT, pq, chunks[0])
                    emit_epilogue(chunks[0], pq, pbc)

                v1 = selpool.tile([128, 8], mybir.dt.float32, tag="v1")
                nc.vector.max(v1[:, :], cands[:, :])
                candr = selpool.tile([128, 64], mybir.dt.float32, tag="candr")
                nc.vector.match_replace(candr[:, :], v1[:, :], cands[:, :], -1e30)
                v2 = selpool.tile([128, 8], mybir.dt.float32, tag="v2")
                nc.vector.max(v2[:, :], candr[:, :])
                bt = selpool.tile([128, 1], mybir.dt.float32, tag="bt")
                nc.vector.tensor_scalar(bt[:, :], v2[:, 7:8], -SCALE, 37.0,
                                        op0=mybir.AluOpType.mult,
                                        op1=mybir.AluOpType.add)

                mask8 = maskpool.tile([128, MK], mybir.dt.float8e4, tag="mask8")
                # banks 4-7 masked in-iteration straight from PSUM (ACT),
                # per chunk so bank c frees as soon as its mask is done
                for c in range(4, NCH):
                    nc.scalar.activation(mask8[:, CH * c:CH * (c + 1)],
                                         chunks[c][:, :],
                                         mybir.ActivationFunctionType.Sigmoid,
                                         bias=bt[:, 0:1], scale=SCALE)

                prev = (rcopy, v2, bt, mask8, q, bc)

            # drain: final iteration's lag-1 masks + last two combines
            prcopy, pv2, pbt, pmask8, ppq, ppbc = prev
            nc.vector.tensor_scalar(pmask8[:, 0:MSPLIT], prcopy[:, 0:MSPLIT],
                                    pv2[:, 7:8], None,
                                    op0=mybir.AluOpType.is_ge)
            nc.scalar.activation(pmask8[:, MSPLIT:2048],
                                 prcopy[:, MSPLIT:2048],
                                 mybir.ActivationFunctionType.Sigmoid,
                                 bias=pbt[:, 0:1], scale=SCALE)
            pm16 = pmask8[:, :].bitcast(mybir.dt.float16)
            mT = maskpool.tile([128, 2048], mybir.dt.float16, tag="maskT")
            nc.sync.dma_start_transpose(
                mT[:, :].rearrange("p (t b) -> p t b", t=16), pm16[:, :])
            pend.append((mT, ppq, ppbc))

            for di, (pmT, pq, pbc) in enumerate(pend):
                wp_last = psum.tile([128, CH], mybir.dt.float32,
                                    tag=f"bank{di}", name=f"bankd{di}")
                emit_mm2(pmT, pq, wp_last)
                emit_epilogue(wp_last, pq, pbc)

            nc.sync.dma_start(out=w_d.ap(), in_=stage[:, :])
    nc.compile()
    return nc


def _get():
    if "k" not in _cache:
        _cache["k"] = _build()
    return _cache["k"]


def _fixup_rows(W, cnt, x, K, M):
    """Recompute rows whose on-device selection count != 16 with the exact
    reference formula (fp32)."""
    bad = np.argwhere(np.abs(cnt - 16.0) > 0.25)
    if len(bad) == 0:
        return W
    xf = np.asarray(x, np.float32)
    Kf = np.asarray(K, np.float32)
    Mf = np.asarray(M, np.float32)
    for b, q in bad:
        xb = xf[b]
        xb = xb / max(np.sqrt(np.sum(xb * xb)), 1e-12)
        Kq = Kf[q]
        nrm = np.maximum(np.sqrt(np.sum(Kq * Kq, axis=1)), 1e-12)
        r = (Kq @ xb) / nrm
        idx = np.argsort(-r, kind="stable")[:DELTA]
        tr = r[idx]
        a = np.exp(S_TEMP * (tr - tr.max()))
        a /= a.sum()
        W[b, q] = (a[:, None] * Mf[q][idx]).sum(0)
    return W


def _run_spmd(nc, in_maps, trace):
    try:
        return run_bass_kernel_spmd(nc, in_maps, core_ids=list(range(N_CORES)),
                                    trace=trace)
    except Exception:
        return run_bass_kernel_spmd(nc, in_maps, core_ids=list(range(N_CORES)),
                                    trace=trace)


def _run(x, K, M, trace=False):
    x = np.ascontiguousarray(np.asarray(x, np.float32))
    K = np.ascontiguousarray(np.asarray(K, np.float32))
    M = np.ascontiguousarray(np.asarray(M, np.float32))

    # host layout glue: f16 cast of M with the uniform 1/16 weight folded
    # in, count column at 1/16, pair interleave
    M16 = (M.astype(np.float32) / 16.0).astype(np.float16)
    ones = np.full((MK, 1), 1.0 / 16.0, np.float16)
    Mp = np.stack([
        np.concatenate([M16[q], ones], 1)
        .reshape(16, 128, 2, U1).transpose(1, 0, 2, 3).reshape(128, 32 * U1)
        for q in range(Q)])

    nc = _get()
    in_maps = []
    for c in range(N_CORES):
        in_maps.append({
            "xc": x,
            "Kc": np.ascontiguousarray(K[c * QS:(c + 1) * QS]),
            "Mp": np.ascontiguousarray(Mp[c * QS:(c + 1) * QS]),
        })
    res = _run_spmd(nc, in_maps, trace)
    # stage[p, (q*G+bc)*U1 + u]: batch row b = bc*128 + p
    Ws, cnts = [], []
    for r in res.results:
        st = np.asarray(r["WS"], np.float32).reshape(128, QS, G, U1)
        Wc = st[:, :, :, :64].transpose(2, 0, 1, 3).reshape(BF, QS, 64)
        cc = (st[:, :, :, 64] * 16.0).transpose(2, 0, 1).reshape(BF, QS)
        Ws.append(Wc)
        cnts.append(cc)
    W = np.concatenate(Ws, axis=1)
    cnt = np.concatenate(cnts, axis=1)

    W = _fixup_rows(W, cnt, x, K, M)
    return W, res.exec_time_ns or 0, 0


def kernel(x, K, M):
    W, _, _ = _run(x, K, M, trace=False)
    return W

